# revision 19
# baseline (speedup 1.0000x reference)
"""Trainium2 Bass kernel for DecoderAttnRNN (LSTM + attention decoder).

Sharding: 8 cores = 4 batch-groups x 2 vocab-halves. Each core: 16 batches,
16000 vocab columns.

Device structure (phases interleaved for PE utilization):
  phase 0: xw = x@W_ih.T + bias (embeddings pre-gathered/transposed on host)
  phase 1: 72-step LSTM recurrence, features-on-partitions layout.
           Single-tanh trick: host pre-scales i,o,f rows by 0.5 so
           sigmoid(a) = 0.5*tanh(a/2)+0.5; state is 2*c / 2*h (host folds
           0.5 into W_hh h-columns, W_lin h-half, and the exp scale).
  phase 2: attention in t-chunks, batched across b, normalize-attn-first.
  phase 3: logits GEMM (1152 x 512 x 16000 bf16) as closed (mi, ng)
           accumulation groups interleaved into phase-1 step gaps;
           PSUM->SBUF evac on ScalarE; bf16 output; b_lin added on host.
"""

import numpy as np
import ml_dtypes

import concourse.bass as bass
import concourse.mybir as mybir
import concourse.tile as tile
from concourse import bacc
from concourse.bass_utils import run_bass_kernel_spmd

B, T, S, E, H, V = 64, 72, 72, 128, 256, 32000
NCORES = 8
NBG = 4                   # batch groups
NVH = 2                   # vocab halves
BL = B // NBG             # 16 batches per core
BT = BL * T               # 1152 rows, t-major: row = t*BL + b
VL = V // NVH             # 16000 vocab cols per core
G4H = 4 * H               # 1024
NCH = G4H // 128          # 8 gate chunks of 128
NMT = BT // 128           # 9 m-tiles (8 timesteps each)
TCM = 24                  # max attention chunk (timesteps)
NGC = 1000                # vocab cols per GEMM group (2 psum banks f32)
NGN = VL // NGC           # 16 groups
UNITS_PER_STEP = 2        # full GEMM groups injected per LSTM step

f32 = mybir.dt.float32
bf16 = mybir.dt.bfloat16
i32 = mybir.dt.int32

_CACHE = {}


def _build():
    nc = bacc.Bacc(None, target_bir_lowering=False)

    xT_d = nc.declare_dram_parameter("xT", [E, BT], bf16, isOutput=False)
    enc_d = nc.declare_dram_parameter("enc", [S, BL, H], bf16, isOutput=False)
    encT_d = nc.declare_dram_parameter("encT", [2, 128, BL, S], bf16, isOutput=False)
    h0T_d = nc.declare_dram_parameter("h0T", [128, 2, BL], bf16, isOutput=False)
    c0T_d = nc.declare_dram_parameter("c0T", [128, 2, BL], f32, isOutput=False)
    lens_d = nc.declare_dram_parameter("lens", [BL], i32, isOutput=False)
    biasT_d = nc.declare_dram_parameter("biasT", [128, NCH], f32, isOutput=False)
    wihT_d = nc.declare_dram_parameter("wihT", [E, G4H], bf16, isOutput=False)
    whhT_d = nc.declare_dram_parameter("whhT", [2, 128, G4H], bf16, isOutput=False)
    wlinT_d = nc.declare_dram_parameter("wlinT", [4, 128, VL], bf16, isOutput=False)
    out_d = nc.declare_dram_parameter("logits", [BT, VL], bf16, isOutput=True)

    exp_scale = float(0.5 / np.sqrt(H))

    with tile.TileContext(nc) as tc:
        with (
            tc.tile_pool(name="persist", bufs=1) as pp,
            tc.tile_pool(name="step", bufs=2) as sp,
            tc.tile_pool(name="attn", bufs=2) as ap_,
            tc.tile_pool(name="oute", bufs=3) as op_,
            tc.tile_pool(name="pg", bufs=1, space="PSUM") as pgp,
            tc.tile_pool(name="p3", bufs=2, space="PSUM") as p3p,
            tc.tile_pool(name="pa", bufs=1, space="PSUM") as pap,
            tc.tile_pool(name="pa2", bufs=2, space="PSUM") as pap2,
        ):
            # ---- persistent tiles ----
            # DMA queue plan: critical path (xw + step 0) spread over
            # gpsimd+scalar; bulk (enc/encT then 16MB wlin) on sync so the
            # fabric isn't hogged before the first recurrence steps.
            x_allT = pp.tile([128, BT], bf16)
            nc.gpsimd.dma_start(out=x_allT[:], in_=xT_d[:])
            wih_sb = pp.tile([128, G4H], bf16)
            nc.scalar.dma_start(out=wih_sb[:], in_=wihT_d[:])
            biasT_sb = pp.tile([128, NCH], f32)
            nc.scalar.dma_start(out=biasT_sb[:], in_=biasT_d[:])
            whh_sb = pp.tile([128, 2, G4H], bf16)
            for k in range(2):
                nc.scalar.dma_start(out=whh_sb[:, k, :], in_=whhT_d[k])

            # z01 holds hhat (=2h), slot t+1 for step t; slot 0 = 2*h0
            z01 = pp.tile([128, 2, T + 1, BL], bf16)
            nc.gpsimd.dma_start(out=z01[:, :, 0, :], in_=h0T_d[:])
            chat = pp.tile([128, 2, BL], f32)      # 2*c state
            nc.gpsimd.dma_start(out=chat[:], in_=c0T_d[:])
            z23 = pp.tile([128, 2, T, BL], bf16)   # ctx features

            encT_sb = pp.tile([128, 2, BL, S], bf16)
            nc.scalar.dma_start(out=encT_sb[:, 0], in_=encT_d[0])
            nc.gpsimd.dma_start(out=encT_sb[:, 1], in_=encT_d[1])
            enc_sb = pp.tile([S, BL, H], bf16)
            nc.gpsimd.dma_start(out=enc_sb[:], in_=enc_d[:])

            # mask stored [S, TCM, BL] (DMA fastest dim contiguous);
            # read transposed where [s, b, t] is needed
            lens_i = pp.tile([S, TCM, BL], i32)
            lens_bcast = bass.AP(
                tensor=lens_d, offset=0, ap=[[0, S], [0, TCM], [1, BL]]
            )
            nc.gpsimd.dma_start(out=lens_i[:], in_=lens_bcast)
            lens_f = pp.tile([S, TCM, BL], f32)
            nc.vector.tensor_copy(out=lens_f[:], in_=lens_i[:])
            iota_i = pp.tile([S, 1], i32)
            nc.gpsimd.iota(iota_i[:], [[1, 1]], base=0, channel_multiplier=1)
            iota_f = pp.tile([S, 1], f32)
            nc.vector.tensor_copy(out=iota_f[:], in_=iota_i[:])
            maskb = pp.tile([S, TCM, BL], bf16)
            nc.vector.tensor_scalar(
                out=maskb[:], in0=lens_f[:], scalar1=iota_f[:], scalar2=None,
                op0=mybir.AluOpType.is_gt,
            )

            ones_col = pp.tile([S, 1], bf16)
            nc.vector.memset(ones_col[:], 1.0)
            ones_s = pp.tile([1, S], f32)
            nc.vector.memset(ones_s[:], 1.0)

            wlin_sb = pp.tile([128, 4, VL], bf16)
            xwT = pp.tile([128, T, NCH, BL], bf16)

            # ---- phase 0: xw = x@W_ih.T + biasT (per 24-timestep group) ----
            NTG = 3
            for tg in range(NTG):
                n0 = tg * 384
                for c in range(NCH):
                    ps_xw = pap2.tile([128, 384], f32, tag="ctxps")
                    nc.tensor.matmul(
                        ps_xw[:],
                        wih_sb[:, c * 128 : (c + 1) * 128],
                        x_allT[:, n0 : n0 + 384],
                        start=True,
                        stop=True,
                    )
                    nc.vector.tensor_scalar(
                        out=xwT[:, tg * 24 : (tg + 1) * 24, c, :],
                        in0=ps_xw[:].rearrange("p (t b) -> p t b", b=BL),
                        scalar1=biasT_sb[:, c : c + 1],
                        scalar2=None,
                        op0=mybir.AluOpType.add,
                    )

            # W_lin preload is emitted inside the step loop (piece-major so
            # the first GEMM groups' slices land first) — late program order
            # keeps Tile's consolidated waits for early PE work clean of the
            # 16MB bulk's DMA-queue ticks.
            WCH = 4000
            wlin_chunks = [(k, p) for p in range(VL // WCH) for k in range(4)]

            def emit_wlin(n):
                for _ in range(n):
                    if not wlin_chunks:
                        return
                    k, p = wlin_chunks.pop(0)
                    nc.sync.dma_start(
                        out=wlin_sb[:, k, p * WCH : (p + 1) * WCH],
                        in_=wlinT_d[k][:, p * WCH : (p + 1) * WCH],
                    )

            zsrc = [
                z01[:, 0, 1:, :].rearrange("p t b -> p (t b)"),
                z01[:, 1, 1:, :].rearrange("p t b -> p (t b)"),
                z23[:, 0].rearrange("p t b -> p (t b)"),
                z23[:, 1].rearrange("p t b -> p (t b)"),
            ]

            # ---- phase 3: one closed GEMM group = (mi, ng) ----
            work_q = []

            def gemm_group(mi, ng):
                def run():
                    # 512-padded per n-chunk so each matmul output stays
                    # within one PSUM bank (start=True clears whole banks)
                    po = p3p.tile([128, 2, 512], f32, tag="po")
                    for k in range(4):
                        for n in range(2):
                            nc.tensor.matmul(
                                po[:, n, :500],
                                zsrc[k][:, mi * 128 : (mi + 1) * 128],
                                wlin_sb[:, k, ng * NGC + n * 500 :
                                        ng * NGC + (n + 1) * 500],
                                start=(k == 0),
                                stop=(k == 3),
                            )
                    ob = op_.tile([128, NGC], bf16, tag="ob")
                    if (mi + ng) % 2 == 0:
                        nc.scalar.copy(
                            out=ob[:].rearrange("p (a b) -> p a b", a=2),
                            in_=po[:, :, :500],
                        )
                    else:
                        nc.vector.tensor_copy(
                            out=ob[:].rearrange("p (a b) -> p a b", a=2),
                            in_=po[:, :, :500],
                        )
                    nc.gpsimd.dma_start(
                        out=out_d[mi * 128 : (mi + 1) * 128,
                                  ng * NGC : (ng + 1) * NGC],
                        in_=ob[:],
                    )
                return run

            def pump(n):
                for _ in range(n):
                    if not work_q:
                        return
                    work_q.pop(0)()

            # ---- phase 2: attention for t in [t0, t0+tcl), batched over b ----
            def attention_chunk(t0, tcl):
                ps_s = pap.tile([S, BL, tcl], f32, tag="attnps")
                for b in range(BL):
                    for k in range(2):
                        nc.tensor.matmul(
                            ps_s[:, b, :],
                            encT_sb[:, k, b, :],
                            z01[:, k, 1 + t0 : 1 + t0 + tcl, b],
                            start=(k == 0),
                            stop=(k == 1),
                        )
                expsc = ap_.tile([S, BL, tcl], bf16, tag="expsc")
                nc.scalar.activation(
                    out=expsc[:], in_=ps_s[:],
                    func=mybir.ActivationFunctionType.Exp,
                    scale=exp_scale,
                )
                nc.vector.tensor_tensor(
                    out=expsc[:], in0=expsc[:],
                    in1=maskb[:, :tcl, :].rearrange("s t b -> s b t"),
                    op=mybir.AluOpType.mult,
                )
                ps_d = pap.tile([1, BL * tcl], f32, tag="attnps")
                nc.tensor.matmul(
                    ps_d[:], ones_col[:],
                    expsc[:].rearrange("s b t -> s (b t)"),
                    start=True, stop=True,
                )
                recip = ap_.tile([1, BL * tcl], f32, tag="recip")
                nc.vector.reciprocal(out=recip[:], in_=ps_d[:])
                ps_bc = pap.tile([S, BL * tcl], f32, tag="attnps")
                nc.tensor.matmul(
                    ps_bc[:], ones_s[:], recip[:], start=True, stop=True
                )
                attn = ap_.tile([S, BL, tcl], bf16, tag="attw")
                nc.vector.tensor_tensor(
                    out=attn[:],
                    in0=expsc[:],
                    in1=ps_bc[:].rearrange("s (b t) -> s b t", b=BL),
                    op=mybir.AluOpType.mult,
                )
                for j in range(2):
                    ps_c = pap2.tile([128, BL, tcl], f32, tag="ctxps")
                    for b in range(BL):
                        nc.tensor.matmul(
                            ps_c[:, b, :],
                            enc_sb[:, b, j * 128 : (j + 1) * 128],
                            attn[:, b, :],
                            start=True, stop=True,
                        )
                    nc.scalar.copy(
                        out=z23[:, j, t0 : t0 + tcl, :],
                        in_=ps_c[:].rearrange("p b t -> p t b"),
                    )

            # attention every 8 steps: smooth GEMM supply of 2 groups/step
            attn_plan = {8 * k + 7: (8 * k, 8) for k in range(NMT)}

            # ---- phase 1: LSTM recurrence with interleaved fillers ----
            # host gate order (i, o, f, g): chunks 0-1=i, 2-3=o, 4-5=f, 6-7=g
            for t in range(T):
                ps_g = pgp.tile([128, NCH, BL], f32, tag="psg")
                for c in range(NCH):
                    for k in range(2):
                        nc.tensor.matmul(
                            ps_g[:, c, :],
                            whh_sb[:, k, c * 128 : (c + 1) * 128],
                            z01[:, k, t, :],
                            start=(k == 0),
                            stop=(k == 1),
                        )
                gates = sp.tile([128, NCH, BL], bf16, tag="gates")
                nc.vector.tensor_tensor(
                    out=gates[:], in0=ps_g[:], in1=xwT[:, t],
                    op=mybir.AluOpType.add,
                )
                tg_t = sp.tile([128, NCH, BL], bf16, tag="tg")
                nc.scalar.activation(
                    out=tg_t[:], in_=gates[:],
                    func=mybir.ActivationFunctionType.Tanh,
                )
                av = sp.tile([128, 6, BL], bf16, tag="av")
                nc.vector.tensor_scalar(
                    out=av[:, 0:4, :], in0=tg_t[:, 0:4, :],
                    scalar1=1.0, scalar2=None, op0=mybir.AluOpType.add,
                )
                nc.vector.tensor_scalar(
                    out=av[:, 4:6, :], in0=tg_t[:, 4:6, :],
                    scalar1=0.5, scalar2=0.5,
                    op0=mybir.AluOpType.mult, op1=mybir.AluOpType.add,
                )
                m_t = sp.tile([128, 2, BL], f32, tag="m")
                nc.vector.tensor_tensor(
                    out=m_t[:], in0=av[:, 4:6, :], in1=chat[:],
                    op=mybir.AluOpType.mult,
                )
                n_t = sp.tile([128, 2, BL], f32, tag="n")
                nc.vector.tensor_tensor(
                    out=n_t[:], in0=av[:, 0:2, :], in1=tg_t[:, 6:8, :],
                    op=mybir.AluOpType.mult,
                )
                nc.vector.tensor_tensor(
                    out=chat[:], in0=m_t[:], in1=n_t[:], op=mybir.AluOpType.add
                )
                tcs = sp.tile([128, 2, BL], bf16, tag="tc")
                nc.scalar.activation(
                    out=tcs[:], in_=chat[:],
                    func=mybir.ActivationFunctionType.Tanh, scale=0.5,
                )
                nc.vector.tensor_tensor(
                    out=z01[:, :, t + 1, :], in0=av[:, 2:4, :], in1=tcs[:],
                    op=mybir.AluOpType.mult,
                )

                if t in attn_plan:
                    t0, tcl = attn_plan[t]
                    attention_chunk(t0, tcl)
                    for mi in range(t0 // 8, (t0 + tcl) // 8):
                        for ng in range(NGN):
                            work_q.append(gemm_group(mi, ng))
                emit_wlin(2)
                pump(UNITS_PER_STEP)

            pump(len(work_q))
    nc.compile()
    return nc


def _prep_inputs(inputs):
    bf = ml_dtypes.bfloat16
    target = np.asarray(inputs["target_tensor"])
    enc = np.asarray(inputs["encoder_outputs"], dtype=np.float32)
    lens = np.asarray(inputs["encoder_seq_lens"])
    h0 = np.asarray(inputs["h0"], dtype=np.float32)
    c0 = np.asarray(inputs["c0"], dtype=np.float32)
    emb = np.asarray(inputs["emb"], dtype=np.float32)
    W_ih = np.asarray(inputs["W_ih"], dtype=np.float32)
    W_hh = np.asarray(inputs["W_hh"], dtype=np.float32)
    bias = (
        np.asarray(inputs["b_ih"], dtype=np.float32)
        + np.asarray(inputs["b_hh"], dtype=np.float32)
    )
    # gate order (i, f, g, o) -> (i, o, f, g)
    perm = np.concatenate(
        [np.arange(0, H), np.arange(3 * H, 4 * H),
         np.arange(H, 2 * H), np.arange(2 * H, 3 * H)]
    )
    W_ih = W_ih[perm]
    W_hh = W_hh[perm]
    bias = bias[perm]
    # tanh trick: sigmoid(a) = 0.5*tanh(a/2)+0.5 -> halve i,o,f pre-activations
    rowscale = np.ones((G4H, 1), np.float32)
    rowscale[: 3 * H] = 0.5
    # state is hhat = 2h: halve W_hh's h-input columns
    W_ih = W_ih * rowscale
    W_hh = W_hh * rowscale * 0.5
    bias = bias * rowscale[:, 0]

    W_lin = np.asarray(inputs["W_lin"], dtype=np.float32)
    wlinT_full = W_lin.T.copy()                                   # (512, V)
    wlinT_full[:H] *= 0.5                                         # hhat = 2h

    wihT = np.ascontiguousarray(W_ih.T.astype(bf))                # (E, 4H)
    whhT = np.ascontiguousarray(W_hh.T.reshape(2, 128, G4H).astype(bf))
    biasT = np.ascontiguousarray(bias.reshape(NCH, 128).T)        # (128, NCH)
    wlinT_bf = wlinT_full.astype(bf)

    in_maps = []
    for i in range(NCORES):
        bg = i % NBG
        vh = i // NBG
        sl = slice(bg * BL, (bg + 1) * BL)
        vsl = slice(vh * VL, (vh + 1) * VL)
        # host embedding gather, t-major columns, transposed: (E, BT)
        tok = target[sl].T.reshape(BT)
        xT = np.ascontiguousarray(emb[tok].astype(bf).T)
        enc_i = enc[sl]                                           # (BL, S, H)
        enc_sbh = np.ascontiguousarray(enc_i.transpose(1, 0, 2).astype(bf))
        encT = np.ascontiguousarray(
            enc_i.transpose(2, 0, 1).reshape(2, 128, BL, S).astype(bf)
        )
        h0T = np.ascontiguousarray(
            (2.0 * h0[sl]).T.reshape(2, 128, BL).transpose(1, 0, 2).astype(bf)
        )
        c0T = np.ascontiguousarray(
            (2.0 * c0[sl]).T.reshape(2, 128, BL).transpose(1, 0, 2)
        )
        wlinT = np.ascontiguousarray(wlinT_bf[:, vsl].reshape(4, 128, VL))
        in_maps.append(
            {
                "xT": xT,
                "enc": enc_sbh,
                "encT": encT,
                "h0T": h0T,
                "c0T": c0T,
                "lens": np.ascontiguousarray(lens[sl].astype(np.int32)),
                "biasT": biasT,
                "wihT": wihT,
                "whhT": whhT,
                "wlinT": wlinT,
            }
        )
    return in_maps


LAST_RESULTS = None


def _install_ntff_shim():
    """Provide antenv.axon_hooks if the image's antenv lacks it, so
    trace=True/BASS_TRACE=1 can capture NTFF profiles under axon."""
    import sys
    import types

    try:
        from antenv.axon_hooks import get_axon_ntff_profile_hook  # noqa: F401

        return
    except ImportError:
        pass
    try:
        from trn_agent_boot.trn_boot import _ntff_profile_via_ctypes

        hook = _ntff_profile_via_ctypes("/opt/axon/libaxon_pjrt.so")
        m = types.ModuleType("antenv.axon_hooks")
        m.get_axon_ntff_profile_hook = lambda: hook
        m.set_axon_ntff_profile_hook = lambda h: None
        sys.modules["antenv.axon_hooks"] = m
    except Exception:
        pass


def kernel(**inputs):
    global LAST_RESULTS
    _install_ntff_shim()
    if "nc" not in _CACHE:
        _CACHE["nc"] = _build()
    nc = _CACHE["nc"]
    in_maps = _prep_inputs(inputs)
    res = run_bass_kernel_spmd(nc, in_maps, core_ids=list(range(NCORES)))
    LAST_RESULTS = res
    b_lin = np.asarray(inputs["b_lin"], dtype=np.float32)
    out = np.empty((B, T, V), dtype=np.float32)
    for i in range(NCORES):
        bg = i % NBG
        vh = i // NBG
        vsl = slice(vh * VL, (vh + 1) * VL)
        # logits rows are t-major: row = t*BL + b
        blk = (
            res.results[i]["logits"]
            .reshape(T, BL, VL)
            .transpose(1, 0, 2)
            .astype(np.float32)
        )
        blk += b_lin[vsl]
        out[bg * BL : (bg + 1) * BL, :, vsl] = blk
    return out
